# revision 48
# baseline (speedup 1.0000x reference)
"""BiLSTM-CRF loss kernel for 8 Trainium2 NeuronCores — single merged NEFF.

Strategy (v5):
  One SPMD program, 8 cores; core c owns sequences [16c, 16c+16).

  LSTM: each core runs SIX interleaved chains — (fwd, bwd) x 3 time
  segments, warm-started 8 steps early (the forget gate contracts
  state influence, so a zero-state warm start carries ~1e-4 relative
  error, far below the bf16 noise floor).  Three (fwd, bwd) pairs give
  the Activation engine ~3 pair-steps of work per serial step path, so
  it stays ~96% busy — the LSTM phase is Act-bound.  Gates live in a
  TRANSPOSED layout [128 partitions (gate-unit) x 8 chunks x 2 members
  x 16 batch]: per-step matmuls stream only N=16 columns, and ONE
  fused Tanh per pair covers sigmoid(i,f,o) + tanh(g) of BOTH
  members (i,f,o rows pre-halved; bwd members' ih matmuls are split
  per step so both members' gates share the psum step column).  All
  elementwise ops (a, u, th, c, h) cover both members at once; the c
  update runs on the otherwise-idle Pool engine.  The pairs are
  emitted a third of a step out of phase so each pair's serial
  act->elementwise->matmul path hides under the other pairs' engine
  work.  Emissions accumulate in PSUM and are flushed to SBUF.

  CRF: the scan is linear in exp space, so it splits into a forward
  alpha scan (t: 0..255) and a backward gamma scan (t: 511..256).
  Each direction is further split into NH=8 concurrent sub-scans:
  exp(trans) is strictly positive, so the scan step contracts the
  Hilbert projective metric (~0.5x per step; diagonal emission
  scalings are isometries) — a sub-scan started from ones converges
  in DIRECTION within 8 warmup steps to ~1e-2 (reaching the loss only
  as ~3e-6 relative error through the final dot), and its magnitude is
  fixed EXACTLY by ratios of component sums captured at the handoff
  steps (applied in log space on the host).  Each direction advances
  all 8 sub-scans with ONE matmul (halves moving together in one psum
  tile) and ONE fused DVE multiply per round — 39 serial rounds
  total.  The Act engine exponentiates emission chunks ahead of the
  scan; the transition matrix is pre-scaled so no rescaling is needed
  at S=512.  The gold-path emission sum runs as outer-product psum
  accumulation on the otherwise-idle PE; its diagonal is extracted
  once at the end.  The host adds the tag-indexed
  transition/start/end/b_out terms and the final logs.
"""
import numpy as np
import ml_dtypes

import bass_rust
import concourse.bass as bass
import concourse.tile as tile
from concourse import mybir
from concourse.bass_utils import run_bass_kernel_spmd
from bass_rust import ScopedClock

f32 = mybir.dt.float32
bf16 = mybir.dt.bfloat16
i32 = mybir.dt.int32
P = 128

V, E, H, T = 50000, 256, 256, 50
B, S = 128, 512
H4 = 4 * H
N = 16            # sequences per core
NSUB = 8          # sequences per CRF subchain (GPSIMD can't read PSUM, so
                  # the scan multiplies stay on DVE; 2 subchains minimize
                  # the per-instruction PSUM-access overhead)
NCORES = 8
NPAIR = 3         # (fwd, bwd) chain pairs; 3 hides the ~1.6us serial
                  # step path under the Act engine's per-slot work
IHB = 4           # steps per ih-matmul block
EMB = 4           # steps per emission psum block (6 chains x 4 x N f32
                  # fits one PSUM bank)
CHK = 16          # CRF steps per exp chunk
RESC = 64         # CRF rescale period
MSH = 6           # log2 pre-scale folded into exp(trans)
bfnp = ml_dtypes.bfloat16
A = mybir.AluOpType
ACT = mybir.ActivationFunctionType


# ---------------------------------------------------------------------------
# TileContext subclass: this toolchain's walrus rejects >1 sync wait per
# instruction; split extra waits into single-wait NoOp prefixes (and do the
# same for the kernel-tail drain's global waits).
# ---------------------------------------------------------------------------
class _TC(tile.TileContext):
    def _split_waits(self, inst):
        si = getattr(inst, "sync_info", None)
        if si is None or not si.on_wait or len(si.on_wait) <= 1:
            return []
        if inst.engine == mybir.EngineType.Unassigned:
            return []
        waits = list(si.on_wait)
        si.on_wait = waits[-1:]
        nops = []
        for w in waits[:-1]:
            nops.append(bass_rust.InstNoOp(
                text_hint="wsplit", bass_nofuse=True,
                name=self.nc.get_next_instruction_name(),
                engine=inst.engine, ins=[], outs=[],
                sync_info=mybir.SyncInfo(on_wait=[w], on_update=[]),
            ))
        return nops

    def _add_instruction(self, inst):
        for n in self._split_waits(inst):
            super()._add_instruction(n)
        super()._add_instruction(inst)

    def _drain_and_barrier(self, tick_clock, wait_clock):
        nc = self.nc
        probe = nc.sync.nop(hint="tail_wait_probe", nofuse=True)
        wait_clock.add_sem_waits(probe.ins,
                                 ScopedClock({None: tick_clock.global_clock}))
        si = probe.ins.sync_info
        waits = list(si.on_wait) if si is not None else []
        if si is not None:
            si.on_wait = waits[:1]
        for w in waits[1:]:
            n2 = nc.sync.nop(hint="tail_wait", nofuse=True)
            si2 = n2.ins.sync_info
            if si2 is None:
                n2.ins.sync_info = mybir.SyncInfo(on_wait=[w], on_update=[])
            else:
                si2.on_wait = [w]
        nc.sync.drain()
        nc.all_engine_barrier()
        popped = nc._tile_sem_poison_stack.pop()
        assert popped is self._sem_poison
        nc.clear_and_free_semaphores(list(self.sems.allocated().values()))
        nc.all_engine_barrier()


# ---------------------------------------------------------------------------
# The merged kernel
# ---------------------------------------------------------------------------
def build(steps=S):
    nc = bass.Bass()
    SN = steps * N
    n_blk = steps // IHB
    n_chk = steps // CHK

    xsT_d = nc.declare_dram_parameter("xsT", [P, 2, SN], bf16, isOutput=False)
    w_d = {}
    for d in (0, 1):
        sfx = "f" if d == 0 else "b"
        w_d[d] = dict(
            wihT=nc.declare_dram_parameter(f"wihT_{sfx}", [P, 2, H4], bf16,
                                           isOutput=False),
            whhT=nc.declare_dram_parameter(f"whhT_{sfx}", [P, 2, H4], bf16,
                                           isOutput=False),
            biasT=nc.declare_dram_parameter(f"biasT_{sfx}", [1, H4], bf16,
                                            isOutput=False),
            woutT=nc.declare_dram_parameter(f"woutT_{sfx}", [P, 2, T], bf16,
                                            isOutput=False),
        )
    etr_d = nc.declare_dram_parameter("etrans", [T, T], bf16, isOutput=False)
    etrT_d = nc.declare_dram_parameter("etransT", [T, T], bf16, isOutput=False)
    bout_d = nc.declare_dram_parameter("bout", [T, 1], f32, isOutput=False)
    estart_d = nc.declare_dram_parameter("estart", [T, 1], f32, isOutput=False)
    eend_d = nc.declare_dram_parameter("eend", [T, 1], f32, isOutput=False)
    oh_d = nc.declare_dram_parameter("oh", [T, SN], bf16, isOutput=False)
    id16_d = nc.declare_dram_parameter("ident16", [N, N], f32, isOutput=False)
    out_d = nc.declare_dram_parameter("out", [1, 31 * N], f32, isOutput=True)

    with _TC(nc) as tc:
        with (
            tc.tile_pool(name="wp", bufs=1) as wp,
            tc.tile_pool(name="xp", bufs=1) as xp,
            tc.tile_pool(name="ep", bufs=1) as ep,
            tc.tile_pool(name="st", bufs=1) as stp,
            tc.tile_pool(name="hp", bufs=2) as hp,
            tc.tile_pool(name="sc", bufs=3) as sc,
            tc.tile_pool(name="es", bufs=4) as esp,
            tc.tile_pool(name="ee", bufs=4) as eep,
            tc.tile_pool(name="psG", bufs=2, space="PSUM") as psG,
            tc.tile_pool(name="psE", bufs=2, space="PSUM") as psEp,
        ):
            # ---- weights / constants
            W = {}
            for d in (0, 1):
                W[d] = dict(
                    wihT=wp.tile([P, 2, H4], bf16, name=f"wihT{d}"),
                    whhT=wp.tile([P, 2, H4], bf16, name=f"whhT{d}"),
                    biasT=wp.tile([1, H4], bf16, name=f"biasT{d}"),
                    woutT=wp.tile([P, 2, T], bf16, name=f"woutT{d}"),
                )
            etr = wp.tile([T, T], bf16, name="etr")
            etrT = wp.tile([T, T], bf16, name="etrT")
            bout = wp.tile([T, 1], f32, name="bout")
            estart = wp.tile([T, 1], f32, name="estart")
            eend = wp.tile([T, 1], f32, name="eend")
            ones50c = wp.tile([T, 1], f32, name="ones50c")
            ones50r = wp.tile([1, T], f32, name="ones50r")
            onesN = wp.tile([1, IHB * N], bf16, name="onesN")
            nc.gpsimd.memset(ones50c[:], 1.0)
            nc.gpsimd.memset(ones50r[:], 1.0)
            nc.gpsimd.memset(onesN[:], 1.0)

            oh = ep.tile([T, SN], bf16, name="oh")
            id16 = wp.tile([N, N], f32, name="id16")
            ones16c = wp.tile([N, 1], f32, name="ones16c")
            nc.gpsimd.memset(ones16c[:], 1.0)

            # ---- startup DMA order: unblock pair-0 chains (xs chunks 0/7,
            # fwd+bwd ih/hh weights) as early as possible, then the other
            # pairs' chunks, then everything the CRF needs later
            xsT = xp.tile([P, 2, SN], bf16, name="xsT")
            n_xc = 8
            xc = SN // n_xc

            def xdma(i):
                nc.gpsimd.dma_start(xsT[:, :, i * xc:(i + 1) * xc],
                                    xsT_d[:, :, i * xc:(i + 1) * xc])

            # three DMA queues in parallel: fwd weights on SP, bwd
            # weights on the (idle) Act queue, xs chunks on gpsimd; the
            # two mid-sequence xs chunks ride SP so pairs 1/2 unblock
            # before the gpsimd queue reaches them
            nc.sync.dma_start(W[0]["wihT"][:], w_d[0]["wihT"][:])
            nc.scalar.dma_start(W[1]["wihT"][:], w_d[1]["wihT"][:])
            xdma(0)
            nc.sync.dma_start(W[0]["biasT"][:], w_d[0]["biasT"][:])
            nc.scalar.dma_start(W[1]["biasT"][:], w_d[1]["biasT"][:])
            xdma(7)
            nc.sync.dma_start(W[0]["whhT"][:], w_d[0]["whhT"][:])
            nc.scalar.dma_start(W[1]["whhT"][:], w_d[1]["whhT"][:])
            nc.sync.dma_start(W[0]["woutT"][:], w_d[0]["woutT"][:])
            nc.scalar.dma_start(W[1]["woutT"][:], w_d[1]["woutT"][:])
            nc.sync.dma_start(xsT[:, :, 2 * xc:3 * xc],
                              xsT_d[:, :, 2 * xc:3 * xc])
            nc.sync.dma_start(xsT[:, :, 5 * xc:6 * xc],
                              xsT_d[:, :, 5 * xc:6 * xc])
            for i in (1, 6, 3, 4):
                xdma(i)
            nc.sync.dma_start(etrT[:], etrT_d[:])
            nc.sync.dma_start(etr[:], etr_d[:])
            nc.sync.dma_start(bout[:], bout_d[:])
            nc.sync.dma_start(estart[:], estart_d[:])
            nc.sync.dma_start(eend[:], eend_d[:])
            nc.sync.dma_start(id16[:], id16_d[:])
            # oh is needed only by the CRF numerator; it lands mid-LSTM,
            # long before use
            nc.scalar.dma_start(oh[:], oh_d[:])

            # ---- emission buffers (full, on-core)
            e_f = ep.tile([T, steps, N], f32, name="e_f")
            e_b = ep.tile([T, steps, N], f32, name="e_b")
            e_dir = (e_f, e_b)

            # ---- 6 LSTM chains: (dir, time-segment) with warmup.  The
            # forget gate contracts state influence, so a segment started
            # from a zero state 16 steps early carries ~1e-4 relative state
            # error, far below the bf16 noise floor.  Chains are paired per
            # segment (fwd+bwd) sharing a psum gate tile; segment keep-spans
            # are 8-aligned so emission flush blocks stay aligned.
            WARM = 16
            kp = (steps + 2 * WARM + NPAIR * 8 - 1) // (NPAIR * 8) * 8
            segs = []      # (s0, length, warm) per segment
            pos0 = 0
            for seg in range(NPAIR):
                warm = 0 if seg == 0 else WARM
                keep = min(kp - warm, steps - pos0)
                segs.append((pos0 - warm, keep + warm, warm))
                pos0 += keep
            assert pos0 == steps
            chains = []
            for seg in range(NPAIR):
                s0, length, warm = segs[seg]
                for d in (0, 1):
                    chains.append(dict(
                        ci=len(chains), d=d, seg=seg, pi=seg,
                        s0=s0, length=length, warm=warm))
            LMAX = max(ch["length"] for ch in chains)

            # per-pair state tile [P, chunk, member, N]: chunks 0..7 gate
            # tanh outputs (both members, written by ONE fused act), 8..9 c
            stP = {}
            hT = {}
            for pi in range(NPAIR):
                stP[pi] = stp.tile([P, 10, 2, N], bf16, name=f"st{pi}")
                nc.gpsimd.memset(stP[pi][:, 8:10, :, :], 0.0)
                # pair h tile [P, eh, member, N]
                h0 = hp.tile([P, 2, 2, N], bf16, tag=f"hT{pi}",
                             name=f"hT{pi}_init")
                nc.gpsimd.memset(h0[:], 0.0)
                hT[pi] = h0

            psg = [{} for _ in range(NPAIR)]  # per pair: blk -> psum tile
            pse = [None]                      # shared emission psum
            pend = [[] for _ in range(NPAIR)]  # per pair: pending ih jobs
            pair_nblk = [chains[2 * pi]["length"] // 2
                         for pi in range(NPAIR)]

            def xcol(ch, k):
                """xs column start for a chain's global step k."""
                return k * N if ch["d"] == 0 else (steps - 1 - k) * N

            def make_ih_jobs(pi, blk):
                # gate psum for 2 steps x 2 pair members: [128, 8, 2, 2, 16].
                # sg indexes the CHAIN-LOCAL step parity for BOTH members (so
                # one fused act can read pg[:, :, :, pos, :]); bwd members'
                # ih matmuls are therefore split per step, since their xs
                # columns descend as the chain step ascends.
                pg = psG.tile([P, 8, 2, 2, N], f32, tag=f"ih{pi}",
                              name=f"pg{pi}_{blk}")
                psg[pi][blk] = pg
                jobs = []
                for m, ch in enumerate(chains[2 * pi:2 * pi + 2]):
                    k0 = ch["s0"] + 2 * blk
                    d = ch["d"]
                    for eh in (0, 1):
                        for j in range(8):
                            if d == 0:
                                cx = xcol(ch, k0)
                                def jf(eh=eh, j=j, pg=pg, m=m, cx=cx, d=d):
                                    nc.tensor.matmul(
                                        pg[:, j, m, :, :],
                                        W[d]["wihT"][:, eh, j * P:(j + 1) * P],
                                        xsT[:, eh, cx:cx + 2 * N],
                                        start=(eh == 0), stop=False,
                                        skip_group_check=True)
                                jobs.append(jf)
                            else:
                                for sg in (0, 1):
                                    cx = xcol(ch, k0 + sg)
                                    def jf(eh=eh, j=j, pg=pg, m=m, cx=cx,
                                           d=d, sg=sg):
                                        nc.tensor.matmul(
                                            pg[:, j, m, sg, :],
                                            W[d]["wihT"][:, eh,
                                                         j * P:(j + 1) * P],
                                            xsT[:, eh, cx:cx + N],
                                            start=(eh == 0), stop=False,
                                            skip_group_check=True)
                                    jobs.append(jf)
                    for j in range(8):
                        def jb(j=j, pg=pg, m=m, d=d):
                            nc.tensor.matmul(
                                pg[:, j, m, :, :],
                                W[d]["biasT"][:, j * P:(j + 1) * P],
                                onesN[:, 0:2 * N],
                                start=False, stop=False,
                                skip_group_check=True)
                        jobs.append(jb)
                return jobs

            def emit_emis(ch, l):
                # deferred emission for local step l (hT[pi] still holds it)
                ci, d = ch["ci"], ch["d"]
                kl = l - ch["warm"]
                g = kl % EMB if d == 0 else EMB - 1 - kl % EMB
                for eh in (0, 1):
                    nc.tensor.matmul(pse[0][:, ci, g, :],
                                     W[d]["woutT"][:, eh, :],
                                     hT[ci // 2][:, eh, ci % 2, :],
                                     start=(eh == 0), stop=(eh == 1),
                                     skip_group_check=True)

            def emit_psE_copy(ch, l):
                # flush the psum block holding kept steps [l-7 .. l] (Pool)
                ci, d = ch["ci"], ch["d"]
                k0 = ch["s0"] + (l - EMB + 1)          # global k of group 0
                t0 = k0 if d == 0 else steps - 1 - (k0 + EMB - 1)
                nc.vector.tensor_copy(
                    e_dir[d][:, t0:t0 + EMB, :], pse[0][:, ci, :, :])

            for pi in range(NPAIR):
                for j in make_ih_jobs(pi, 0):
                    j()
                pend[pi] = make_ih_jobs(pi, 1)

            def flush_pair(pi, l):
                # flush emissions for pair pi's chains at an EMB boundary
                for ch in chains[2 * pi:2 * pi + 2]:
                    if l > ch["warm"] and l <= ch["length"]:
                        emit_emis(ch, l - 1)
                        emit_psE_copy(ch, l - 1)

            def phase_a(pi, l):
                # whh matmuls (both members), ih dribble, ONE fused gate act
                blk, pos = divmod(l, 2)
                pg = psg[pi][blk]
                if pend[pi]:
                    n_do = (len(pend[pi]) + 1 - pos) // (2 - pos)
                    for _ in range(n_do):
                        pend[pi].pop(0)()
                for m, ch in enumerate(chains[2 * pi:2 * pi + 2]):
                    ci, d = ch["ci"], ch["d"]
                    if l % EMB != 0 and l > ch["warm"]:
                        emit_emis(ch, l - 1)
                    for eh in (0, 1):
                        for j in range(8):
                            nc.tensor.matmul(
                                pg[:, j, m, pos, :],
                                W[d]["whhT"][:, eh, j * P:(j + 1) * P],
                                hT[pi][:, eh, m, :],
                                start=False, stop=(eh == 1),
                                skip_group_check=True)
                # W = tanh(G): one act for sig(i,f,o) of BOTH members
                # (pre-halved rows) and tanh(g)
                nc.scalar.activation(stP[pi][:, 0:8, :, :],
                                     pg[:, :, :, pos, :], ACT.Tanh)
                if pos == 1:
                    assert not pend[pi]
                    if blk + 2 < pair_nblk[pi]:
                        pend[pi] = make_ih_jobs(pi, blk + 2)
                    psg[pi].pop(blk)

            def phase_b(pi, l):
                # elementwise, each op covering BOTH members at once
                s = stP[pi]
                # A = (t_if + 1) * [tg | c]
                a_t = sc.tile([P, 4, 2, N], bf16, tag=f"A{pi}",
                              name=f"A{pi}_{l}")
                nc.vector.scalar_tensor_tensor(
                    a_t[:], s[:, 0:4, :, :], 1.0, s[:, 6:10, :, :],
                    op0=A.add, op1=A.mult)
                # u = A_i + A_f (= 2*c_new)
                u_t = sc.tile([P, 2, 2, N], bf16, tag=f"u{pi}",
                              name=f"u{pi}_{l}")
                nc.vector.tensor_tensor(u_t[:], a_t[:, 0:2, :, :],
                                        a_t[:, 2:4, :, :], op=A.add)
                # th = tanh(c) = tanh(0.5*u), both members in one act
                th = sc.tile([P, 2, 2, N], bf16, tag=f"th{pi}",
                             name=f"th{pi}_{l}")
                nc.scalar.activation(th[:], u_t[:], ACT.Tanh, scale=0.5)
                # c state (Pool, off the critical path)
                nc.gpsimd.tensor_scalar(out=s[:, 8:10, :, :], in0=u_t[:],
                                        scalar1=0.5, scalar2=None,
                                        op0=A.mult)
                # h2 = (t_o+1) * th == 2h; whh/wout halved on host
                hn = hp.tile([P, 2, 2, N], bf16, tag=f"hT{pi}",
                             name=f"hT{pi}_{l}")
                nc.vector.scalar_tensor_tensor(
                    hn[:], s[:, 4:6, :, :], 1.0, th[:],
                    op0=A.add, op1=A.mult)
                hT[pi] = hn

            # The pairs are emitted a third of a step out of phase so each
            # pair's serial act->elementwise->matmul path hides under the
            # other pairs' engine work.
            plen = [chains[2 * pi]["length"] for pi in range(NPAIR)]

            # scan carrier init tiles: memset the warm-half ones NOW so the
            # Pool queue isn't doing it on the scan-start critical path
            inis = []
            for pr in (0, 1):
                ini = sc.tile([T, 8, N], bf16, tag=f"cc{pr}",
                              name=f"ini{pr}")
                nc.gpsimd.memset(ini[:, 1:8, :], 1.0)
                inis.append(ini)

            def a_ok(pi, l):
                return 0 <= l < plen[pi]

            for l in range(LMAX + 1):
                if l % EMB == 0:
                    flush_pair(0, l)
                    flush_pair(1, l)
                if a_ok(0, l):
                    phase_a(0, l)
                if a_ok(2, l - 1):
                    phase_b(2, l - 1)
                if l % EMB == 0:
                    flush_pair(2, l)
                if a_ok(1, l):
                    phase_a(1, l)
                if a_ok(0, l):
                    phase_b(0, l)
                if a_ok(2, l):
                    phase_a(2, l)
                if a_ok(1, l):
                    phase_b(1, l)
                if l % EMB == 0 and l < LMAX:
                    pse[0] = psEp.tile([T, 2 * NPAIR, EMB, N], f32, tag="em",
                                       name=f"pse_{l}")

            # ---------------------------------------------------------------
            # CRF: exp-space forward algorithm, each direction split into
            # two concurrent sub-scans.  exp(trans) is strictly positive,
            # so the scan step contracts the Hilbert projective metric
            # (~0.5x per step; diagonal emission scalings are isometries):
            # a sub-scan started from ones converges in DIRECTION within
            # WARMC steps to ~1e-7, and its unknown magnitude is fixed
            # exactly by the ratio of component sums captured at the
            # handoff step (both sub-scans summed at the same t; the ratio
            # is applied in log space on the host).  Each pair of
            # same-direction sub-scans shares a psum tile, one matmul
            # (both halves moving together) and one fused DVE multiply
            # per round.
            # ---------------------------------------------------------------
            WARMC = 24
            NH = 8                                # sub-scans per direction
            HALF = (steps // 2 + (NH - 1) * WARMC) // NH   # rounds each
            OFF = HALF - WARMC                    # per-half time offset
            n_ch = (HALF + CHK - 1) // CHK

            ones50b = wp.tile([T, 1], bf16, name="ones50b")
            nc.gpsimd.memset(ones50b[:], 1.0)

            out_sb = wp.tile([1, 31 * N], f32, name="out_sb")
            eacc = wp.tile([1, N], i32, name="eacc")
            nc.gpsimd.memset(eacc[:], 0)

            # numerator: accumulate sum_t oh_t (x) es_t into a [N, N] psum
            # on the otherwise-idle PE; its diagonal is sum_t es[tag_t].
            # (reuses an "em" buffer: that tag is idle after the LSTM)
            psN = psEp.tile([N, N], f32, tag="em", name="psN")

            es_t = {}
            ee_t = {}
            nm_ct = [0]
            nm_q = []

            def chunk_t0(pair, c, h):
                Gc = min(CHK, HALF - CHK * c)
                if pair == 0:
                    return CHK * c + h * OFF
                return (steps - CHK * c - Gc) - h * OFF

            def queue_num(pair, c, Gc):
                es = es_t[(pair, c)]
                for q in range(Gc):
                    for h in range(NH):
                        t = chunk_t0(pair, c, h) + q
                        # half h owns [h*OFF (+WARMC if h>0), h*OFF+HALF)
                        m = t if pair == 0 else steps - 1 - t
                        own = (m >= h * OFF + (WARMC if h > 0 else 0)
                               and m < h * OFF + HALF)
                        if not own:
                            continue

                        def job(t=t, q=q, h=h, es=es):
                            nc.tensor.matmul(
                                psN[:], oh[:, t * N:(t + 1) * N],
                                es[:, q, h, :],
                                start=(nm_ct[0] == 0),
                                stop=(nm_ct[0] == steps - 1),
                                skip_group_check=True)
                            nm_ct[0] += 1
                        nm_q.append(job)

            def emit_chunk_pair(pair, c, exp=True, adds=True):
                Gc = min(CHK, HALF - CHK * c)
                es = esp.tile([T, CHK, NH, N], bf16, tag="es",
                              name=f"es{pair}_{c}")
                ee = eep.tile([T, CHK, NH, N], bf16, tag="ee",
                              name=f"ee{pair}_{c}")
                es_t[(pair, c)] = es
                ee_t[(pair, c)] = ee
                if adds:
                    for h in range(NH):
                        t0 = chunk_t0(pair, c, h)
                        nc.gpsimd.tensor_tensor(
                            es[:, 0:Gc, h, :], e_f[:, t0:t0 + Gc, :],
                            e_b[:, t0:t0 + Gc, :], op=A.add)
                if exp:
                    nc.scalar.activation(ee[:, 0:Gc, :, :],
                                         es[:, 0:Gc, :, :],
                                         ACT.Exp, bias=bout[:, 0:1])
                queue_num(pair, c, Gc)
                return Gc

            def add_part(pair, g0, g1):
                # half 0 last: it alone waits the final LSTM flush
                es = es_t[(pair, 0)]
                for h in range(NH - 1, -1, -1):
                    t0 = chunk_t0(pair, 0, h)
                    nc.gpsimd.tensor_tensor(
                        es[:, g0:g1, h, :], e_f[:, t0 + g0:t0 + g1, :],
                        e_b[:, t0 + g0:t0 + g1, :], op=A.add)

            def exp_part(pair, c, g0, g1):
                nc.scalar.activation(ee_t[(pair, c)][:, g0:g1, :, :],
                                     es_t[(pair, c)][:, g0:g1, :, :],
                                     ACT.Exp, bias=bout[:, 0:1])

            # chunk 0: build and exponentiate the first-needed groups of
            # BOTH pairs first (alpha reads groups ascending, gamma
            # descending) so round 0 gates on ~2us of post-flush work
            # instead of the full 8us of adds + 2x1.9us exps
            Ga = emit_chunk_pair(0, 0, exp=False, adds=False)
            Gb = emit_chunk_pair(1, 0, exp=False, adds=False)
            add_part(0, 0, 4)
            add_part(1, Gb - 4, Gb)
            exp_part(0, 0, 0, 4)
            exp_part(1, 0, Gb - 4, Gb)
            add_part(0, 4, Ga)
            add_part(1, 0, Gb - 4)
            exp_part(0, 0, 4, 10)
            exp_part(1, 0, Gb - 10, Gb - 4)
            exp_part(0, 0, 10, Ga)
            exp_part(1, 0, 0, Gb - 10)

            etrs = (etr, etrT)
            cars = [None, None]
            for pr in (0, 1):
                # half0 carry: the true boundary start; warm halves were
                # memset to ones before the LSTM
                ini = inis[pr]
                g0 = (CHK - 1) if pr == 1 else 0
                ev = estart if pr == 0 else eend
                nc.vector.tensor_scalar(
                    out=ini[:, 0, :], in0=ee_t[(pr, 0)][:, g0, 0, :],
                    scalar1=ev[:, 0:1], scalar2=None, op0=A.mult)
                cars[pr] = ini
                # round 0 advances only the warm halves (half0 starts at
                # its true boundary emission)
                pa0 = psG.tile([T, 2 * NH - 1, N], f32, tag=f"ih{pr}",
                               name=f"pa{pr}_0")
                nc.tensor.matmul(pa0[:, 1:NH, :], etrs[pr][:],
                                 ini[:, 1:NH, :], start=True, stop=True)
                nc.vector.tensor_tensor(
                    ini[:, 1:NH, :], pa0[:, 1:NH, :],
                    ee_t[(pr, 0)][:, g0, 1:NH, :], op=A.mult)

            for r in range(1, HALF):
                c, p = divmod(r, CHK)
                if p == 8 and c + 1 < n_ch:
                    emit_chunk_pair(0, c + 1)
                    emit_chunk_pair(1, c + 1)
                for pr in (0, 1):
                    Gc = min(CHK, HALF - CHK * c)
                    g = p if pr == 0 else Gc - 1 - p
                    pa_t = psG.tile([T, 2 * NH - 1, N], f32, tag=f"ih{pr}",
                                    name=f"pa{pr}_{r}")
                    nc.tensor.matmul(pa_t[:, 0:NH, :], etrs[pr][:],
                                     cars[pr][:, 0:NH, :],
                                     start=True, stop=True)
                    anew = sc.tile([T, NH, N], bf16, tag=f"cc{pr}",
                                   name=f"c{pr}_{r}")
                    nc.vector.tensor_tensor(anew[:], pa_t[:, 0:NH, :],
                                            ee_t[(pr, c)][:, g, :, :],
                                            op=A.mult)
                    cars[pr] = anew
                    # handoff sums: warm halves after warmup, true halves
                    # at range end (same t); the ratios fix magnitudes
                    if r == WARMC - 1 or r == HALF - 1:
                        hs = range(1, NH) if r == WARMC - 1 \
                            else range(0, NH - 1)
                        b0 = 3 if r == WARMC - 1 else 3 + 2 * (NH - 1)
                        for j, h in enumerate(hs):
                            nc.tensor.matmul(pa_t[0:1, NH + j, :],
                                             ones50b[:], anew[:, h, :],
                                             start=True, stop=True)
                            k = b0 + pr * (NH - 1) + j
                            nc.vector.tensor_copy(
                                out_sb[0:1, k * N:(k + 1) * N],
                                pa_t[0:1, NH + j, :])
                for _ in range(10):
                    if nm_q:
                        nm_q.pop(0)()

            for job in nm_q:
                job()
            nm_q.clear()

            # ---- finalize.  The numerator-diagonal path depends only on
            # psN (complete a round early), so it is emitted FIRST and
            # overlaps the scan's last round; the logZ dot
            # (etr^T a'_255) . g'_256 follows.
            dg = sc.tile([N, N], f32, tag="dg", name="dg")
            nc.vector.tensor_tensor(dg[:], psN[:], id16[:], op=A.mult)
            f1 = psG.tile([T, 3, N], f32, tag="ih0", name="fin1")
            nc.tensor.matmul(f1[0:1, 0, :], ones16c[:], dg[:],
                             start=True, stop=True)
            nc.vector.tensor_copy(out_sb[0:1, 0:N], f1[0:1, 0, :])
            nc.vector.tensor_copy(out_sb[0:1, 2 * N:3 * N], eacc[:])
            fz = psG.tile([T, 3, N], f32, tag="ih0", name="finz")
            nc.tensor.matmul(fz[:, 0, :], etr[:], cars[0][:, NH - 1, :],
                             start=True, stop=True)
            gd = sc.tile([T, N], f32, tag="cg", name="gdot")
            nc.vector.tensor_tensor(gd[:], fz[:, 0, :],
                                    cars[1][:, NH - 1, :], op=A.mult)
            nc.tensor.matmul(f1[0:1, 1, :], ones50c[:], gd[:],
                             start=True, stop=True)
            nc.vector.tensor_copy(out_sb[0:1, N:2 * N], f1[0:1, 1, :])
            nc.sync.dma_start(out_d[:], out_sb[:])
    return nc


def emit_rescale(nc, sc, rs_t, ones50c, ones50r, eacc, a, h, t):
    """Divide a by 2^e (e = exponent of sum(a)), add e to eacc[0:N].

    Both the alpha and gamma chains cover all 16 sequences; their shift
    counts sum in the same eacc slots (logZ adds them anyway)."""
    tg = "ca" if h == 0 else "cg"
    ps = rs_t[0:1, 2, :]
    nc.tensor.matmul(ps, ones50c[:], a[:], start=True, stop=True)
    bits = sc.tile([1, N], i32, tag=f"rb{h}", name=f"rb{h}_{t}")
    nc.vector.tensor_copy(bits[:].bitcast(f32), ps)
    # biased exponent (the -127 bias is subtracted on the host; mixing a
    # shift with an arithmetic add in one tensor_scalar is rejected)
    e_t = sc.tile([1, N], i32, tag=f"re{h}", name=f"re{h}_{t}")
    nc.vector.tensor_scalar(e_t[:], bits[:], 23, None,
                            op0=A.arith_shift_right)
    nc.vector.tensor_tensor(eacc[0:1, :], eacc[0:1, :], e_t[:], op=A.add)
    # m = 2^-e as f32 bits: 0x7F000000 - (bits & 0x7F800000)
    #   = ((bits & 0x7F800000) ^ -1) + 0x7F000001
    m1 = sc.tile([1, N], i32, tag=f"rm{h}", name=f"rm{h}_{t}")
    nc.vector.tensor_scalar(m1[:], bits[:], 0x7F800000, -1,
                            op0=A.bitwise_and, op1=A.bitwise_xor)
    m2 = sc.tile([1, N], i32, tag=f"rn{h}", name=f"rn{h}_{t}")
    nc.vector.tensor_scalar(m2[:], m1[:], 0x7F000001, None, op0=A.add)
    pb = rs_t[:, 2, :]
    nc.tensor.matmul(pb, ones50r[:], m2[:].bitcast(f32),
                     start=True, stop=True)
    ar = sc.tile([T, N], f32, tag=tg, name=f"ar{h}_{t}")
    nc.vector.tensor_tensor(ar[:], pb, a[:], op=A.mult)
    return ar


# ---------------------------------------------------------------------------
# Host-side input preparation
# ---------------------------------------------------------------------------
# gate chunk order i,f,o,g; i,f,o rows pre-halved (sigmoid via tanh)
_PERM = np.concatenate([np.arange(0, 512), np.arange(768, 1024),
                        np.arange(512, 768)])
_RS = np.ones((H4, 1), dtype=np.float32)
_RS[0:768] = 0.5


def _ktile(a):
    """[K, N] -> [128, K//128, N] with k = eh*128 + p."""
    K, Nn = a.shape
    return np.ascontiguousarray(a.reshape(K // P, P, Nn).transpose(1, 0, 2))


def prep_inputs(inputs, steps=S):
    sent = np.asarray(inputs["sentences"])
    emb = np.asarray(inputs["embedding"], dtype=np.float32)
    tags = np.asarray(inputs["tags"])[:, :steps]
    trans = np.asarray(inputs["trans"], dtype=np.float32)
    b_out = np.asarray(inputs["b_out"], dtype=np.float32)
    start_trans = np.asarray(inputs["start_trans"], dtype=np.float32)
    end_trans = np.asarray(inputs["end_trans"], dtype=np.float32)
    xs = emb[sent[:, :steps]]                    # [B, steps, E]

    scl = 1.0 / float(np.exp(trans).sum(axis=1).mean())
    etr_np = (np.exp(trans) * scl).astype(np.float32)
    shared = {"etrans": etr_np.astype(bfnp),
              "etransT": np.ascontiguousarray(etr_np.T).astype(bfnp),
              "bout": b_out.reshape(T, 1),
              "estart": np.exp(start_trans).astype(np.float32).reshape(T, 1),
              "eend": np.exp(end_trans).astype(np.float32).reshape(T, 1)}
    wd = {}
    for d, sfx in ((0, "f"), (1, "b")):
        wih = np.asarray(inputs[f"w_ih_{sfx}"], dtype=np.float32)[_PERM] * _RS
        whh = (np.asarray(inputs[f"w_hh_{sfx}"], dtype=np.float32)[_PERM]
               * _RS * 0.5)
        bias = (np.asarray(inputs[f"b_ih_{sfx}"], dtype=np.float32)
                + np.asarray(inputs[f"b_hh_{sfx}"],
                             dtype=np.float32))[_PERM] * _RS[:, 0]
        wout_half = np.asarray(inputs["W_out"],
                               dtype=np.float32)[:, d * H:(d + 1) * H] * 0.5
        wd[d] = {f"wihT_{sfx}": _ktile(wih.T.astype(bfnp)),
                 f"whhT_{sfx}": _ktile(whh.T.astype(bfnp)),
                 f"biasT_{sfx}": bias.astype(bfnp).reshape(1, H4),
                 f"woutT_{sfx}": _ktile(wout_half.T.astype(bfnp))}

    in_maps = []
    for core in range(NCORES):
        xs_c = xs[core * N:(core + 1) * N]       # [N, steps, E]
        xr = xs_c.transpose(1, 0, 2).reshape(steps * N, E)
        tags_c = tags[core * N:(core + 1) * N]   # [N, steps]
        ohc = (np.arange(T)[:, None, None]
               == tags_c.T[None, :, :]).astype(np.float32)  # [T, steps, N]
        m = {"xsT": _ktile(xr.T.astype(bfnp)),
             "oh": np.ascontiguousarray(ohc.reshape(T, steps * N)
                                        .astype(bfnp)),
             "ident16": np.eye(N, dtype=np.float32),
             **wd[0], **wd[1], **shared}
        in_maps.append(m)
    return in_maps


def host_numerator_part(inputs, steps=S):
    """start + transitions + end + b_out[tags] part of the gold score."""
    tags = np.asarray(inputs["tags"])[:, :steps]
    trans = np.asarray(inputs["trans"], dtype=np.float32)
    b_out = np.asarray(inputs["b_out"], dtype=np.float32)
    start_trans = np.asarray(inputs["start_trans"], dtype=np.float32)
    end_trans = np.asarray(inputs["end_trans"], dtype=np.float32)
    num = start_trans[tags[:, 0]] + end_trans[tags[:, -1]]
    num = num + trans[tags[:, :-1], tags[:, 1:]].sum(axis=1)
    num = num + b_out[tags].sum(axis=1)
    return num.astype(np.float32)        # [B]


_cache = {}


def _get_nc(steps=S):
    if steps not in _cache:
        _cache[steps] = build(steps)
    return _cache[steps]


def run_phases(inputs, steps=S, trace=False):
    nc = _get_nc(steps)
    in_maps = prep_inputs(inputs, steps)
    res = run_bass_kernel_spmd(nc, in_maps, core_ids=list(range(NCORES)),
                               trace=trace)
    def col(k):
        return np.concatenate([r["out"][0, k * N:(k + 1) * N]
                               for r in res.results])
    num_e, pz, eacc = col(0), col(1), col(2)
    NH = 8
    lr = np.zeros_like(pz)
    for pr in (0, 1):
        for j in range(NH - 1):
            den = col(3 + pr * (NH - 1) + j)
            nm = col(3 + 2 * (NH - 1) + pr * (NH - 1) + j)
            lr = lr + np.log(nm / den)
    trans = np.asarray(inputs["trans"], dtype=np.float32)
    scl = 1.0 / float(np.exp(trans).sum(axis=1).mean())
    # eacc is zero (no rescales); kept for output-format stability
    logz = (np.log(pz) + np.log(2.0) * eacc - (steps - 1) * np.log(scl)
            + lr)
    num_h = host_numerator_part(inputs, steps)
    loss = -np.mean(num_h + num_e - logz)
    extras = {"num_e": num_e, "logz": logz, "num_h": num_h,
              "exec_a": res.exec_time_ns, "exec_b": None}
    return np.float32(loss), extras


def kernel(**inputs):
    loss, _ = run_phases(inputs, steps=S, trace=False)
    return np.asarray(loss, dtype=np.float32)

